# revision 14
# baseline (speedup 1.0000x reference)
"""Trainium2 Bass kernel: grouped MoE expert MLP (nn_ExpertGroup).

Strategy: expert parallelism across 8 NeuronCores. Tokens are sorted by
expert; core e runs expert e's two GEMMs:
    h = relu(x_e @ w_up[e].T) ** 2      (bf16, like the CUDA reference)
    y = h @ w_down[e].T
The host does the (free) token scatter/gather, the bf16 casts, and the
weight transposes so every device-side DMA is contiguous.

Device layout (per core, cap = padded local token count, default 1024):
    xT  (D=1024, cap)  bf16   x_e.T               -> SBUF [128, 8, cap]
    wuT (D=1024, H=2048) bf16 w_up[e].T           -> SBUF [128, 8, 2048]
    wdT (H=2048, D=1024) bf16 w_down[e].T         -> SBUF [128, 16, 1024]
    GEMM1: psum[j,t] = sum_d wuT[d,j].T @ xT[d,t]   (h in [H, T] layout)
    DVE:   relu -> bf16, square -> hsq SBUF [128, 16, cap]
    GEMM2: psum[t,i] = sum_j hsq[j,t].T @ wdT[j,i]  (y in [T, D] layout)
    DVE:   cast fp32 psum -> bf16 y -> DMA out

Built on bacc.Bacc (not raw Bass): Bacc.compile() legalizes semaphore
waits to the TRN2 limit of one wait per instruction (moving matmul waits
onto ldweights and splitting the rest into EventSemaphore instructions).
Raw Bass BIR fails walrus codegen with "Too many sync wait commands".
"""

import numpy as np
import ml_dtypes

import concourse.bass as bass
import concourse.mybir as mybir
import concourse.tile as tile
from concourse import bacc
from concourse.bass_utils import run_bass_kernel_spmd

T, D, H, E = 8192, 1024, 2048, 8
P = 128
N_CORES = 8
FD = 512  # matmul moving free dim (one PSUM bank of fp32)

_PROGRAM_CACHE: dict[int, "bass.Bass"] = {}
LAST_RESULT = None  # BassKernelResults of the most recent run (for harness use)


def _build_program(cap: int) -> "bass.Bass":
    assert cap % FD == 0
    n_d = D // P       # 8  contraction tiles of GEMM1
    n_j = H // P       # 16 H partition tiles
    n_tc = cap // FD   # token chunks (moving operand of GEMM1)
    n_t = cap // P     # token partition tiles (GEMM2 output)
    bf16 = mybir.dt.bfloat16
    f32 = mybir.dt.float32

    nc = bacc.Bacc("TRN2", debug=False, num_devices=N_CORES)
    xT = nc.dram_tensor("xT", [D, cap], bf16, kind="ExternalInput")
    wuT = nc.dram_tensor("wuT", [D, H], bf16, kind="ExternalInput")
    wdT = nc.dram_tensor("wdT", [H, D], bf16, kind="ExternalInput")
    y = nc.dram_tensor("y", [cap, D], bf16, kind="ExternalOutput")

    xT3 = xT[:].rearrange("(po pi) f -> pi po f", pi=P)    # [128, 8, cap]
    wuT3 = wuT[:].rearrange("(po pi) f -> pi po f", pi=P)  # [128, 8, 2048]
    wdT3 = wdT[:].rearrange("(po pi) f -> pi po f", pi=P)  # [128, 16, 1024]
    y3 = y[:].rearrange("(po pi) f -> pi po f", pi=P)      # [128, n_t, 1024]

    with tile.TileContext(nc) as tc:
        with (
            tc.tile_pool(name="big", bufs=1) as big,
            tc.tile_pool(name="outp", bufs=4) as outp,
            tc.tile_pool(name="actp", bufs=4) as actp,
            tc.tile_pool(name="psum", bufs=8, space="PSUM") as psum,
        ):
            xT_sb = big.tile([P, n_d, cap], bf16)
            wuT_sb = big.tile([P, n_d, H], bf16)
            wdT_sb = big.tile([P, n_j, D], bf16)
            hsq_sb = big.tile([P, n_j, cap], bf16)

            # Input DMAs, ordered so the first GEMM1 group's operands land
            # first. Each is ~1MiB, so a single dma_start already spreads
            # across all 16 SDMA engines.
            for c in range(n_tc):
                nc.sync.dma_start(
                    out=xT_sb[:, :, c * FD:(c + 1) * FD],
                    in_=xT3[:, :, c * FD:(c + 1) * FD],
                )
            for c in range(H // FD):
                nc.sync.dma_start(
                    out=wuT_sb[:, :, c * FD:(c + 1) * FD],
                    in_=wuT3[:, :, c * FD:(c + 1) * FD],
                )
            for c in range(n_j // 4):
                nc.sync.dma_start(
                    out=wdT_sb[:, c * 4:(c + 1) * 4, :],
                    in_=wdT3[:, c * 4:(c + 1) * 4, :],
                )

            # GEMM1 + relu^2: hsq[j, t] (token chunk outer so GEMM2 of the
            # first half can start while the second half computes)
            for c in range(n_tc):
                for j in range(n_j):
                    ps = psum.tile([P, FD], f32, tag="ps")
                    for d in range(n_d):
                        nc.tensor.matmul(
                            ps,
                            wuT_sb[:, d, j * P:(j + 1) * P],
                            xT_sb[:, d, c * FD:(c + 1) * FD],
                            start=(d == 0),
                            stop=(d == n_d - 1),
                        )
                    hr = actp.tile([P, FD], bf16, tag="hr")
                    nc.vector.tensor_relu(out=hr, in_=ps)
                    nc.vector.tensor_mul(
                        out=hsq_sb[:, j, c * FD:(c + 1) * FD], in0=hr, in1=hr
                    )

            # GEMM2: y[t, i] = sum_j hsq[j, t].T @ wdT[j, i]
            for t in range(n_t):
                for ic in range(D // FD):
                    ps = psum.tile([P, FD], f32, tag="ps")
                    for j in range(n_j):
                        nc.tensor.matmul(
                            ps,
                            hsq_sb[:, j, t * P:(t + 1) * P],
                            wdT_sb[:, j, ic * FD:(ic + 1) * FD],
                            start=(j == 0),
                            stop=(j == n_j - 1),
                        )
                    yt = outp.tile([P, FD], bf16, tag="yt")
                    nc.vector.tensor_copy(out=yt, in_=ps)
                    nc.sync.dma_start(
                        out=y3[:, t, ic * FD:(ic + 1) * FD], in_=yt
                    )

    nc.compile()
    return nc


def _get_program(cap: int) -> "bass.Bass":
    nc = _PROGRAM_CACHE.get(cap)
    if nc is None:
        nc = _build_program(cap)
        _PROGRAM_CACHE[cap] = nc
    return nc


def kernel(x, num_tokens_per_expert, w_up, w_down, _trace=False):
    global LAST_RESULT
    bf = ml_dtypes.bfloat16
    x = np.asarray(x)
    counts = np.asarray(num_tokens_per_expert).astype(np.int64)
    w_up = np.asarray(w_up)
    w_down = np.asarray(w_down)
    n_tok = x.shape[0]
    assert counts.shape == (E,) and int(counts.sum()) == n_tok
    offsets = np.zeros(E, dtype=np.int64)
    offsets[1:] = np.cumsum(counts)[:-1]

    cap = int(max(FD, -(-int(counts.max()) // FD) * FD))
    nc = _get_program(cap)

    in_maps = []
    for e in range(E):
        cnt, off = int(counts[e]), int(offsets[e])
        xs = np.zeros((cap, D), dtype=bf)
        xs[:cnt] = x[off:off + cnt].astype(bf)
        in_maps.append({
            "xT": np.ascontiguousarray(xs.T),
            "wuT": np.ascontiguousarray(w_up[e].astype(bf).T),
            "wdT": np.ascontiguousarray(w_down[e].astype(bf).T),
        })

    res = run_bass_kernel_spmd(
        nc, in_maps, core_ids=list(range(N_CORES)), trace=_trace
    )
    LAST_RESULT = res

    out = np.zeros((n_tok, D), dtype=x.dtype)
    for e in range(E):
        cnt, off = int(counts[e]), int(offsets[e])
        if cnt:
            out[off:off + cnt] = res.results[e]["y"][:cnt].astype(x.dtype)
    return out


# revision 17
# speedup vs baseline: 1.0341x; 1.0341x over previous
"""Trainium2 Bass kernel: grouped MoE expert MLP (nn_ExpertGroup).

Strategy: expert parallelism across 8 NeuronCores. Tokens are sorted by
expert; core e runs expert e's two GEMMs:
    h = relu(x_e @ w_up[e].T) ** 2      (bf16, like the CUDA reference)
    y = h @ w_down[e].T
The host does the (free) token scatter/gather, the bf16 casts, and the
weight transposes so every device-side DMA is contiguous.

Device layout (per core, cap = padded local token count, default 1024):
    xT  (D=1024, cap)  bf16   x_e.T               -> SBUF [128, 8, cap]
    wuT (D=1024, H=2048) bf16 w_up[e].T           -> SBUF [128, 8, 2048]
    wdT (H=2048, D=1024) bf16 w_down[e].T         -> SBUF [128, 16, 1024]
    GEMM1: psum[j,t] = sum_d wuT[d,j].T @ xT[d,t]   (h in [H, T] layout)
    DVE:   relu -> bf16, square -> hsq SBUF [128, 16, cap]
    GEMM2: psum[t,i] = sum_j hsq[j,t].T @ wdT[j,i]  (y in [T, D] layout)
    DVE:   cast fp32 psum -> bf16 y -> DMA out

Built on bacc.Bacc (not raw Bass): Bacc.compile() legalizes semaphore
waits to the TRN2 limit of one wait per instruction (moving matmul waits
onto ldweights and splitting the rest into EventSemaphore instructions).
Raw Bass BIR fails walrus codegen with "Too many sync wait commands".
"""

import numpy as np
import ml_dtypes

import concourse.bass as bass
import concourse.mybir as mybir
import concourse.tile as tile
from concourse import bacc
from concourse.bass_utils import run_bass_kernel_spmd

T, D, H, E = 8192, 1024, 2048, 8
P = 128
N_CORES = 8
FD = 512  # matmul moving free dim (one PSUM bank of fp32)

_PROGRAM_CACHE: dict[int, "bass.Bass"] = {}
LAST_RESULT = None  # BassKernelResults of the most recent run (for harness use)


def _build_program(cap: int) -> "bass.Bass":
    assert cap % FD == 0
    n_d = D // P       # 8  contraction tiles of GEMM1
    n_j = H // P       # 16 H partition tiles
    n_tc = cap // FD   # token chunks (moving operand of GEMM1)
    n_t = cap // P     # token partition tiles (GEMM2 output)
    bf16 = mybir.dt.bfloat16
    f32 = mybir.dt.float32

    nc = bacc.Bacc("TRN2", debug=False, num_devices=N_CORES)
    xT = nc.dram_tensor("xT", [D, cap], bf16, kind="ExternalInput")
    wuT = nc.dram_tensor("wuT", [D, H], bf16, kind="ExternalInput")
    wdT = nc.dram_tensor("wdT", [H, D], bf16, kind="ExternalInput")
    y = nc.dram_tensor("y", [cap, D], bf16, kind="ExternalOutput")

    xT3 = xT[:].rearrange("(po pi) f -> pi po f", pi=P)    # [128, 8, cap]
    wuT3 = wuT[:].rearrange("(po pi) f -> pi po f", pi=P)  # [128, 8, 2048]
    wdT3 = wdT[:].rearrange("(po pi) f -> pi po f", pi=P)  # [128, 16, 1024]
    y3 = y[:].rearrange("(po pi) f -> pi po f", pi=P)      # [128, n_t, 1024]

    with tile.TileContext(nc) as tc:
        with (
            tc.tile_pool(name="big", bufs=1) as big,
            tc.tile_pool(name="outp", bufs=4) as outp,
            tc.tile_pool(name="actp", bufs=4) as actp,
            tc.tile_pool(name="psum", bufs=7, space="PSUM") as psum,
            tc.tile_pool(name="warmp", bufs=1, space="PSUM") as warmp,
        ):
            xT_sb = big.tile([P, n_d, cap], bf16)
            wuT_sb = big.tile([P, n_d, H], bf16)
            wdT_sb = big.tile([P, n_j, D], bf16)
            hsq_sb = big.tile([P, n_j, cap], bf16)

            # PE warm-up: ~80 dummy matmuls with no DMA dependencies run
            # while the input DMAs stream in. They keep the PE busy through
            # the HAM activity window so the real matmul stream starts at
            # the full 2.4 GHz clock instead of the cold 1.2 GHz.
            warm = big.tile([P, P], bf16)
            nc.gpsimd.memset(warm[:], 0.0)
            wps = warmp.tile([P, P], f32, tag="warm")
            for _ in range(80):
                nc.tensor.matmul(wps, warm[:], warm[:], start=True, stop=True)

            # Input DMAs, ordered so the first GEMM1 group's operands land
            # first (wuT columns for j=0, then the first token chunk). One
            # dma_start already spreads across all 16 SDMA engines, and the
            # HWDGE ring is FIFO, so issue order = arrival order.
            nc.sync.dma_start(out=wuT_sb[:, :, 0:P], in_=wuT3[:, :, 0:P])
            nc.sync.dma_start(
                out=xT_sb[:, :, 0:FD], in_=xT3[:, :, 0:FD]
            )
            nc.sync.dma_start(
                out=wuT_sb[:, :, P:FD], in_=wuT3[:, :, P:FD]
            )
            for c in range(1, H // FD):
                nc.sync.dma_start(
                    out=wuT_sb[:, :, c * FD:(c + 1) * FD],
                    in_=wuT3[:, :, c * FD:(c + 1) * FD],
                )
            for c in range(1, n_tc):
                nc.sync.dma_start(
                    out=xT_sb[:, :, c * FD:(c + 1) * FD],
                    in_=xT3[:, :, c * FD:(c + 1) * FD],
                )
            for c in range(n_j // 4):
                nc.sync.dma_start(
                    out=wdT_sb[:, c * 4:(c + 1) * 4, :],
                    in_=wdT3[:, c * 4:(c + 1) * 4, :],
                )

            # GEMM1 + relu^2: hsq[j, t] (token chunk outer so GEMM2 of the
            # first half can start while the second half computes)
            for c in range(n_tc):
                for j in range(n_j):
                    ps = psum.tile([P, FD], f32, tag="ps")
                    for d in range(n_d):
                        nc.tensor.matmul(
                            ps,
                            wuT_sb[:, d, j * P:(j + 1) * P],
                            xT_sb[:, d, c * FD:(c + 1) * FD],
                            start=(d == 0),
                            stop=(d == n_d - 1),
                        )
                    hr = actp.tile([P, FD], bf16, tag="hr")
                    nc.vector.tensor_relu(out=hr, in_=ps)
                    nc.vector.tensor_mul(
                        out=hsq_sb[:, j, c * FD:(c + 1) * FD], in0=hr, in1=hr
                    )

            # GEMM2: y[t, i] = sum_j hsq[j, t].T @ wdT[j, i]
            for t in range(n_t):
                for ic in range(D // FD):
                    ps = psum.tile([P, FD], f32, tag="ps")
                    for j in range(n_j):
                        nc.tensor.matmul(
                            ps,
                            hsq_sb[:, j, t * P:(t + 1) * P],
                            wdT_sb[:, j, ic * FD:(ic + 1) * FD],
                            start=(j == 0),
                            stop=(j == n_j - 1),
                        )
                    yt = outp.tile([P, FD], bf16, tag="yt")
                    nc.vector.tensor_copy(out=yt, in_=ps)
                    nc.sync.dma_start(
                        out=y3[:, t, ic * FD:(ic + 1) * FD], in_=yt
                    )

    nc.compile()
    return nc


def _get_program(cap: int) -> "bass.Bass":
    nc = _PROGRAM_CACHE.get(cap)
    if nc is None:
        nc = _build_program(cap)
        _PROGRAM_CACHE[cap] = nc
    return nc


def kernel(x, num_tokens_per_expert, w_up, w_down, _trace=False):
    global LAST_RESULT
    bf = ml_dtypes.bfloat16
    x = np.asarray(x)
    counts = np.asarray(num_tokens_per_expert).astype(np.int64)
    w_up = np.asarray(w_up)
    w_down = np.asarray(w_down)
    n_tok = x.shape[0]
    assert counts.shape == (E,) and int(counts.sum()) == n_tok
    offsets = np.zeros(E, dtype=np.int64)
    offsets[1:] = np.cumsum(counts)[:-1]

    cap = int(max(FD, -(-int(counts.max()) // FD) * FD))
    nc = _get_program(cap)

    in_maps = []
    for e in range(E):
        cnt, off = int(counts[e]), int(offsets[e])
        xs = np.zeros((cap, D), dtype=bf)
        xs[:cnt] = x[off:off + cnt].astype(bf)
        in_maps.append({
            "xT": np.ascontiguousarray(xs.T),
            "wuT": np.ascontiguousarray(w_up[e].astype(bf).T),
            "wdT": np.ascontiguousarray(w_down[e].astype(bf).T),
        })

    res = run_bass_kernel_spmd(
        nc, in_maps, core_ids=list(range(N_CORES)), trace=_trace
    )
    LAST_RESULT = res

    out = np.zeros((n_tok, D), dtype=x.dtype)
    for e in range(E):
        cnt, off = int(counts[e]), int(offsets[e])
        if cnt:
            out[off:off + cnt] = res.results[e]["y"][:cnt].astype(x.dtype)
    return out
